# revision 6
# baseline (speedup 1.0000x reference)
"""Expert-parallel MoE kernel for Trainium2 (8 NeuronCores).

Problem: top-2-of-8 MoE layer, H=768, F=3072, T=2048 tokens, fp32.

Sharding: one expert per core. Every core receives the full token set
(replicated activations) plus its own expert's FFN weights, computes the
router on-device (exact fp32), derives its expert's combine weight per token
(top-2 membership + softmax over the two selected logits), runs the expert
FFN over all tokens (float32r matmuls at full PE rate), scales by the
combine weight, and writes a partial [T, H] output. The host unshards by
summing the 8 partial outputs (the sum-combine of the expert-parallel
sharding).
"""

import numpy as np

import concourse.bass as bass
import concourse.mybir as mybir
import concourse.tile as tile
from concourse import bacc
from concourse.bass_utils import run_bass_kernel_spmd

E = 8
H = 768
F = 3072
T = 2048
B, S = 2, 1024
HC = H // 128        # 6 contraction chunks
FC = F // 128        # 24 F chunks
TB = 256             # token block
NBLK = T // TB       # 8
TPB = TB // 128      # token tiles per block (2)
NTT = T // 128       # 16 token tiles
FQ = 4               # F quarters for SBUF-resident hmid
FTQ = FC // FQ       # 6 F tiles per quarter
HH = 2               # H split for GEMM2 psum (2 x 384)
HN = H // HH         # 384

f32 = mybir.dt.float32
f32r = mybir.dt.float32r
AF = mybir.ActivationFunctionType
OP = mybir.AluOpType


def build_nc():
    nc = bacc.Bacc("TRN2", target_bir_lowering=False, debug=False)

    xT = nc.dram_tensor("xT", [128, HC * T], f32r, kind="ExternalInput")
    xTf = nc.dram_tensor("xTf", [128, HC * T], f32, kind="ExternalInput")
    w1T = nc.dram_tensor("w1T", [128, HC * F], f32r, kind="ExternalInput")
    b1c = nc.dram_tensor("b1c", [128, FC], f32, kind="ExternalInput")
    w2T = nc.dram_tensor("w2T", [128, FC * H], f32r, kind="ExternalInput")
    b2row = nc.dram_tensor("b2row", [1, H], f32r, kind="ExternalInput")
    rT = nc.dram_tensor("rT", [128, HC * E], f32, kind="ExternalInput")
    ones_in = nc.dram_tensor("ones_in", [1, 128], f32r, kind="ExternalInput")
    part = nc.dram_tensor("part", [T, H], f32, kind="ExternalOutput")

    with tile.TileContext(nc) as tc:
        with (
            tc.tile_pool(name="wpool", bufs=1) as wpool,
            tc.tile_pool(name="xpool", bufs=2) as xpool,
            tc.tile_pool(name="hpool", bufs=2) as hpool,
            tc.tile_pool(name="ypool", bufs=4) as ypool,
            tc.tile_pool(name="gpool", bufs=1) as gpool,
            tc.tile_pool(name="rpool", bufs=8) as rpool,
            tc.tile_pool(name="ps1", bufs=2, space="PSUM") as ps1,
            tc.tile_pool(name="psy", bufs=4, space="PSUM") as psy,
            tc.tile_pool(name="psr", bufs=2, space="PSUM") as psr,
        ):
            # resident weights
            w1 = wpool.tile([128, HC, F], f32r, tag="w1")
            w2 = wpool.tile([128, FC, H], f32r, tag="w2")
            b1 = wpool.tile([128, FC], f32, tag="b1")
            b2 = wpool.tile([1, H], f32r, tag="b2")
            rw = wpool.tile([128, HC, E], f32, tag="rw")
            ones1 = wpool.tile([1, 128], f32r, tag="ones1")
            nc.sync.dma_start(w1[:], w1T.ap().rearrange("p (c f) -> p c f", c=HC))
            nc.sync.dma_start(w2[:], w2T.ap().rearrange("p (c h) -> p c h", c=FC))
            nc.sync.dma_start(b1[:], b1c.ap())
            nc.sync.dma_start(b2[:], b2row.ap())
            nc.sync.dma_start(rw[:], rT.ap().rearrange("p (c e) -> p c e", c=HC))
            nc.sync.dma_start(ones1[:], ones_in.ap())

            xT3 = xT.ap().rearrange("p (c t) -> p c t", c=HC)
            xTf3 = xTf.ap().rearrange("p (c t) -> p c t", c=HC)

            for blk in range(NBLK):
                tsl = bass.ts(blk, TB)
                xb = xpool.tile([128, HC, TB], f32r, tag="xb")
                nc.sync.dma_start(xb[:], xT3[:, :, tsl])
                xbf = xpool.tile([128, HC, TB], f32, tag="xbf")
                nc.sync.dma_start(xbf[:], xTf3[:, :, tsl])

                # --- router (exact fp32) + top-2 gates for this block ---
                gblk = rpool.tile([128, TPB], f32, tag="gates")
                for tt in range(TPB):
                    lps = psr.tile([128, E], f32, tag="lps")
                    for k in range(HC):
                        nc.tensor.matmul(
                            lps[:],
                            xbf[:, k, bass.ts(tt, 128)],
                            rw[:, k, :],
                            start=(k == 0),
                            stop=(k == HC - 1),
                        )
                    L = rpool.tile([128, E], f32, tag="L")
                    nc.scalar.activation(L[:], lps[:], AF.Copy)
                    m1 = rpool.tile([128, 1], f32, tag="m1")
                    nc.vector.reduce_max(m1[:], L[:], axis=mybir.AxisListType.X)
                    eq1 = rpool.tile([128, E], f32, tag="eq1")
                    nc.vector.tensor_scalar(eq1[:], L[:], m1[:], None, op0=OP.is_ge)
                    msk = rpool.tile([128, E], f32, tag="msk")
                    nc.vector.scalar_tensor_tensor(
                        msk[:], eq1[:], -1e30, L[:], op0=OP.mult, op1=OP.add
                    )
                    m2 = rpool.tile([128, 1], f32, tag="m2")
                    nc.vector.reduce_max(m2[:], msk[:], axis=mybir.AxisListType.X)
                    # own expert is column 0 (host permutes router rows per core)
                    sel = rpool.tile([128, 1], f32, tag="sel")
                    nc.vector.tensor_scalar(sel[:], L[:, 0:1], m2[:], None, op0=OP.is_ge)
                    d = rpool.tile([128, 1], f32, tag="d")
                    nc.vector.tensor_scalar(d[:], m2[:], m1[:], None, op0=OP.subtract)
                    ed = rpool.tile([128, 1], f32, tag="ed")
                    nc.scalar.activation(ed[:], d[:], AF.Exp)
                    den = rpool.tile([128, 1], f32, tag="den")
                    nc.vector.tensor_scalar(den[:], ed[:], 1.0, None, op0=OP.add)
                    rcp = rpool.tile([128, 1], f32, tag="rcp")
                    nc.vector.reciprocal(rcp[:], den[:])
                    tnum = rpool.tile([128, 1], f32, tag="tnum")
                    nc.vector.tensor_scalar(tnum[:], L[:, 0:1], m1[:], None, op0=OP.subtract)
                    en = rpool.tile([128, 1], f32, tag="en")
                    nc.scalar.activation(en[:], tnum[:], AF.Exp)
                    g1 = rpool.tile([128, 1], f32, tag="g1")
                    nc.vector.tensor_mul(g1[:], en[:], rcp[:])
                    nc.vector.tensor_mul(gblk[:, tt : tt + 1], g1[:], sel[:])

                # --- GEMM2 psum tiles for this block ---
                yps = [
                    [
                        psy.tile([128, HN], f32, tag="yps", name=f"yps_{blk}_{tt}_{hh}")
                        for hh in range(HH)
                    ]
                    for tt in range(TPB)
                ]

                # --- FFN: GEMM1 (per F quarter) -> gelu -> GEMM2 accumulate ---
                for q in range(FQ):
                    hq = hpool.tile([128, FTQ, TB], f32r, tag="hq")
                    for ft in range(FTQ):
                        fc = q * FTQ + ft
                        hps = ps1.tile([128, TB], f32, tag="hps")
                        for k in range(HC):
                            nc.tensor.matmul(
                                hps[:],
                                w1[:, k, bass.ts(fc, 128)],
                                xb[:, k, :],
                                start=(k == 0),
                                stop=(k == HC - 1),
                            )
                        nc.scalar.activation(
                            hq[:, ft, :], hps[:], AF.Gelu, bias=b1[:, fc : fc + 1]
                        )
                    for tt in range(TPB):
                        for hh in range(HH):
                            for ft in range(FTQ):
                                fc = q * FTQ + ft
                                nc.tensor.matmul(
                                    yps[tt][hh][:],
                                    hq[:, ft, bass.ts(tt, 128)],
                                    w2[:, fc, bass.ts(hh, HN)],
                                    start=(q == 0 and ft == 0),
                                    stop=False,
                                )
                # bias row (rank-1) closes each accumulation group
                for tt in range(TPB):
                    for hh in range(HH):
                        nc.tensor.matmul(
                            yps[tt][hh][:],
                            ones1[:, :],
                            b2[:, bass.ts(hh, HN)],
                            start=False,
                            stop=True,
                        )

                # --- scale by gate, evict, store ---
                for tt in range(TPB):
                    ysb = ypool.tile([128, H], f32, tag="ysb")
                    for hh in range(HH):
                        nc.vector.tensor_scalar(
                            ysb[:, bass.ts(hh, HN)],
                            yps[tt][hh][:],
                            gblk[:, tt : tt + 1],
                            None,
                            op0=OP.mult,
                        )
                    row0 = blk * TB + tt * 128
                    nc.sync.dma_start(part.ap()[row0 : row0 + 128, :], ysb[:])
    nc.compile()
    return nc


_NC = None


def _get_nc():
    global _NC
    if _NC is None:
        _NC = build_nc()
    return _NC


def _chunk_partition(a, nchunks):
    """[nchunks*128, X] -> [128, nchunks, X] flattened to [128, nchunks*X]."""
    n, x = a.shape
    return np.ascontiguousarray(
        a.reshape(nchunks, 128, x).transpose(1, 0, 2).reshape(128, nchunks * x)
    )


def kernel(hidden_states, router_w, w1, b1, w2, b2):
    nc = _get_nc()
    x = np.asarray(hidden_states, dtype=np.float32).reshape(T, H)
    router_w = np.asarray(router_w, dtype=np.float32)
    w1 = np.asarray(w1, dtype=np.float32)
    b1 = np.asarray(b1, dtype=np.float32)
    w2 = np.asarray(w2, dtype=np.float32)
    b2 = np.asarray(b2, dtype=np.float32)

    xT = _chunk_partition(np.ascontiguousarray(x.T), HC)  # [128, HC*T]

    in_maps = []
    for e in range(E):
        perm = [e] + [j for j in range(E) if j != e]
        rt = _chunk_partition(np.ascontiguousarray(router_w[perm].T), HC)
        w1t = _chunk_partition(np.ascontiguousarray(w1[e].T), HC)  # [H,F]
        w2t = _chunk_partition(np.ascontiguousarray(w2[e].T), FC)  # [F,H]
        b1ce = np.ascontiguousarray(b1[e].reshape(FC, 128).T)
        in_maps.append(
            {
                "xT": xT,
                "xTf": xT,
                "w1T": w1t,
                "b1c": b1ce,
                "w2T": w2t,
                "b2row": b2[e].reshape(1, H),
                "rT": rt,
                "ones_in": np.ones((1, 128), dtype=np.float32),
            }
        )

    global _last_in_maps
    _last_in_maps = in_maps
    res = run_bass_kernel_spmd(nc, in_maps, core_ids=list(range(E)))
    out = np.zeros((T, H), dtype=np.float32)
    for e in range(E):
        out += res.results[e]["part"]
    return out.reshape(B, S, H)
